# revision 1
# baseline (speedup 1.0000x reference)
"""JointFluxSingleTransformerBlockControl — TRN2 Bass kernel, 8-core tensor parallel.

Sharding (per core c of 8):
  - heads: 3 of 24  (q/k/v column-parallel, both streams)
  - mlp hidden: 1536 of 12288 rows
  - ada-norm emb rows: 1152 of 9216 (matvec sharded, per-stream AllGather)
  - out-proj: column-parallel over this core's 1920 h-columns -> partial
    [3072, 1024] (T-layout) per stream; gate, out_b/8 and residual/8 are
    folded in on device so the host does a pure sum over cores.

Layout: activations in T-layout [feature=partition, seq=free]; weights are
pre-transposed on host so no on-device transposes are needed anywhere.

All matmul operands are bf16 (4x tensor-engine throughput vs fp32, half the
weight DMA); accumulation stays fp32 in PSUM. Row-broadcasts (rstd, rms,
softmax denominators) go through a K=1 ones-matmul instead of a DRAM bounce.
The layernorm apply is interleaved with the q-projection so the tensor
engine is not starved behind the DVE; both streams' stats run before the
first apply to cover the AllGather latency. Phase order groups same-
activation-table work (sqrt | gelu | exp) to minimize table reloads.
"""

import numpy as np
import ml_dtypes

import concourse.bass as bass
import concourse.bacc as bacc
import concourse.tile as tile
from concourse import mybir
from concourse.bass_utils import run_bass_kernel_spmd

F32 = mybir.dt.float32
BF16 = mybir.dt.bfloat16
AF = mybir.ActivationFunctionType

D = 3072
S = 1024
HD = 128
NCORES = 8
HPC = 3                  # heads per core
QO = HPC * HD            # 384 q/k/v out-dims per core
MLPC = 12288 // NCORES   # 1536
ES = 9216 // NCORES      # 1152 e-rows per core
KC = D // 128            # 24 contraction chunks
EPS = 1e-6
INV_SQRT_HD = float(1.0 / np.sqrt(128.0))
HKC = (QO + MLPC) // 128  # 15 h-col chunks per core


def bcast(ap, p=128):
    """Partition-broadcast a free-dims-only AP to [p, *free]."""
    return bass.AP(tensor=ap.tensor, offset=ap.offset, ap=[[0, p]] + list(ap.ap))


def build_nc():
    nc = bacc.Bacc(None, target_bir_lowering=False)
    dp = nc.declare_dram_parameter
    I = {}
    for s in ("m", "c"):
        I[f"xT_{s}"] = dp(f"xT_{s}", [D, S], BF16, isOutput=False)
        I[f"temb_{s}"] = dp(f"temb_{s}", [128, KC], BF16, isOutput=False)
        I[f"normT_{s}"] = dp(f"normT_{s}", [D, ES], BF16, isOutput=False)
        I[f"nb_{s}"] = dp(f"nb_{s}", [1, ES], BF16, isOutput=False)
        for w in ("q", "k", "v"):
            I[f"w{w}T_{s}"] = dp(f"w{w}T_{s}", [D, QO], BF16, isOutput=False)
        I[f"qb_{s}"] = dp(f"qb_{s}", [128, HPC], F32, isOutput=False)
        I[f"kb_{s}"] = dp(f"kb_{s}", [128, HPC], F32, isOutput=False)
        I[f"vb_{s}"] = dp(f"vb_{s}", [1, QO], BF16, isOutput=False)
        for pj in ("q", "k"):
            # rope cos/sin with the per-head rms-norm weight folded in:
            # rows 0-127: cos[p]*w[p]; rows 128-255: sin[p]*w[p^1]
            I[f"ropeW_{pj}_{s}"] = dp(f"ropeW_{pj}_{s}", [256, S], BF16,
                                      isOutput=False)
    I["mlpT"] = dp("mlpT", [D, MLPC], BF16, isOutput=False)
    I["mlpb"] = dp("mlpb", [128, MLPC // 128], F32, isOutput=False)
    # outT blocked per output chunk: row (oc*128 + p) holds the 15*128
    # h-contraction line for output dim block oc, partition p -> one
    # contiguous DMA per oc.
    I["outT"] = dp("outT", [KC * 128, HKC * 128], BF16, isOutput=False)
    I["outb"] = dp("outb", [128, KC], F32, isOutput=False)  # chunk-col layout
    I["rotT"] = dp("rotT", [128, 128], BF16, isOutput=False)
    OUT = {
        "m": dp("out_m", [D, S], BF16, isOutput=True),
        "c": dp("out_c", [D, S], BF16, isOutput=True),
    }

    with tile.TileContext(nc) as tc, nc.allow_low_precision("bf16 kernel, 2e-2 gate"):
        with (
            tc.tile_pool(name="dram", bufs=1, space="DRAM") as dram,
            tc.tile_pool(name="const", bufs=1) as const,
        ):
            PS = {}
            ones = const.tile([128, 1], BF16)
            nc.vector.memset(ones, 1.0)
            onesrow = const.tile([1, 128], BF16)
            nc.vector.memset(onesrow, 1.0)
            negrow = const.tile([1, 128], BF16)
            nc.vector.memset(negrow, -1.0)
            epst = const.tile([128, 1], F32)
            nc.vector.memset(epst, EPS)
            qm_arena = const.tile([128, HPC, S], BF16, tag="qm_arena",
                                  name="qm_arena")

            def mm_bcast(row, dst, lhs=None):
                """dst[128, S] (SBUF bf16) = broadcast of row[1, S] (bf16)
                via a K=1 matmul; ACT evacuates PSUM immediately."""
                if lhs is None:
                    lhs = onesrow
                for st in range(2):
                    sl = slice(st * 512, (st + 1) * 512)
                    bp = PS["p"].tile([128, 512], F32, tag="mmT", bufs=2,
                                      name="bc_ps")
                    nc.tensor.matmul(bp, lhs, row[:, sl], start=True, stop=True)
                    nc.scalar.activation(dst[:, sl], bp, AF.Copy)

            # ---------------- phase E + layernorm stats -------------------
            # The ada-norm matvec (PE, PSUM) and the layernorm stats (DVE
            # partial sums + gpsimd partition-reduce, no PSUM) are
            # interleaved chunk-by-chunk so the startup is paced by DMA, not
            # serialized phases.  Stream c's matvec/stats are hooked into
            # stream m's v-projection loops later.
            eb, ag, eps_t, st_t = {}, {}, {}, {}
            xsum, sqsum = {}, {}

            def phase_e_mm(s, kk):
                if kk == 0:
                    st = const.tile([128, KC], BF16, tag=f"silu_{s}",
                                    name="st")
                    nc.sync.dma_start(st, I[f"temb_{s}"][:])
                    st_t[s] = st
                    eps_t[s] = [PS["p"].tile([1, 384], F32, tag="mm", bufs=6,
                                          name="eps_t") for _ in range(3)]
                wn = pe.tile([128, ES], BF16, tag="wnorm", bufs=2, name="wn")
                nc.sync.dma_start(wn, I[f"normT_{s}"][kk * 128:(kk + 1) * 128, :])
                for nt in range(3):
                    nc.tensor.matmul(
                        eps_t[s][nt], st_t[s][:, kk:kk + 1],
                        wn[:, nt * 384:(nt + 1) * 384],
                        start=(kk == 0), stop=(kk == KC - 1))

            def phase_e_finish(s):
                nbr = pe.tile([1, ES], BF16, tag="nbrow", name="nbr")
                nc.sync.dma_start(nbr, I[f"nb_{s}"][:])
                erow = pe.tile([1, ES], F32, tag="erow", name="erow")
                ebt = dram.tile([1, ES], F32, tag=f"eb_{s}", name="ebt")
                for nt in range(3):
                    sl = slice(nt * 384, (nt + 1) * 384)
                    nc.vector.tensor_add(erow[:, sl], eps_t[s][nt],
                                         nbr[:, sl])
                nc.sync.dma_start(ebt, erow)
                ago = dram.tile([NCORES, ES], F32, tag=f"ag_{s}", name="ago")
                nc.gpsimd.collective_compute(
                    "AllGather", mybir.AluOpType.bypass,
                    replica_groups=[list(range(NCORES))],
                    ins=[ebt.opt()], outs=[ago.opt()])
                ag[s] = ago

            def stats_alloc():
                sum_ps = [PS["p"].tile([1, 512], F32, tag="mm", bufs=6,
                                    name="sum_ps") for _ in range(2)]
                sq_ps = [PS["p"].tile([1, 512], F32, tag="mm", bufs=6,
                                   name="sq_ps"),
                         PS["p"].tile([1, 512], F32, tag="mmT", bufs=2,
                                      name="sq_ps2")]
                return sum_ps, sq_ps

            def stats_chunk(s, kk, sps):
                sum_ps, sq_ps = sps
                xk = pn.tile([128, S], BF16, tag=f"xk{kk % 3}", name="xk")
                nc.sync.dma_start(xk,
                                  I[f"xT_{s}"][kk * 128:(kk + 1) * 128, :])
                sq = pn.tile([128, S], BF16, tag=f"t1{kk % 2}", name="sq")
                nc.vector.tensor_mul(sq, xk, xk)
                for st in range(2):
                    sl = slice(st * 512, (st + 1) * 512)
                    nc.tensor.matmul(sum_ps[st], ones, xk[:, sl],
                                     start=(kk == 0), stop=(kk == KC - 1))
                    nc.tensor.matmul(sq_ps[st], ones, sq[:, sl],
                                     start=(kk == 0), stop=(kk == KC - 1))

            def stats_finish(s, sum_ps, sq_ps):
                mu = pstat.tile([1, S], F32, tag="mu", name="mu")
                msq = pstat.tile([1, S], F32, tag="msq", name="msq")
                for st in range(2):
                    sl = slice(st * 512, (st + 1) * 512)
                    nc.scalar.activation(mu[:, sl], sum_ps[st], AF.Copy,
                                         scale=1.0 / D)
                    nc.scalar.activation(msq[:, sl], sq_ps[st], AF.Copy,
                                         scale=1.0 / D)
                var = pstat.tile([1, S], BF16, tag="var", name="var")
                nc.vector.tensor_mul(var, mu, mu)
                nc.vector.tensor_sub(var, msq, var)
                rstd = pstat.tile([1, S], F32, tag="msq", name="rstd")
                nc.scalar.activation(rstd, var, AF.Sqrt, bias=epst[:1, :])
                rstd_row = pstat.tile([1, S], BF16, tag="rstd_row",
                                      name="rstd_row")
                nc.vector.reciprocal(rstd_row, rstd)
                nmr_row = pstat.tile([1, S], BF16, tag="nmr_row",
                                     name="nmr_row")
                nc.vector.tensor_mul(nmr_row, mu, rstd_row)
                mm_bcast(rstd_row, rstd_bc[s])
                mm_bcast(nmr_row, nmr_bc[s], lhs=negrow)  # -mu*rstd

            ss, scale1, g_sb, ob8 = {}, {}, {}, {}

            def sst_unpack(s):
                # ago row c = core c's e-slice; flat index c*ES + jj*128 + p.
                ag3 = ag[s][:].rearrange("c (jj p) -> p c jj", p=128)
                sst = const.tile([128, 48], F32, tag=f"ss_{s}", name="sst")
                nc.sync.dma_start(
                    sst[:, 0:45].rearrange("p (cc j) -> p cc j", j=9),
                    ag3[:, 0:5, :])
                nc.sync.dma_start(sst[:, 45:48], ag3[:, 5, 0:3])
                s1 = const.tile([128, KC], F32, tag=f"s1_{s}", name="s1")
                nc.vector.tensor_scalar_add(s1, sst[:, 24:48], 1.0)
                ss[s], scale1[s] = sst, s1
                gt = const.tile([128, KC], F32, tag=f"gate_{s}", name="gt")
                nc.sync.dma_start(gt[:, 0:6], ag3[:, 5, 3:9])
                nc.sync.dma_start(
                    gt[:, 6:24].rearrange("p (cc j) -> p cc j", j=9),
                    ag3[:, 6:8, :])
                g_sb[s] = gt
                ot = const.tile([128, KC], F32, tag=f"ob8_{s}", name="ot")
                nc.vector.tensor_mul(ot, gt, outb_cc)
                nc.vector.tensor_scalar_mul(ot, ot, 1.0 / NCORES)
                ob8[s] = ot

            # deferred const DMAs (not needed until the q/k head posts /
            # MLP / out-proj; issued as a hook inside the k(m) loop)
            mbt = const.tile([128, MLPC // 128], F32, tag="mlpb")
            outb_cc = const.tile([128, KC], F32, tag="outb_cc")
            qkb = {}
            vbb = {}
            for s in ("m", "c"):
                for pj in ("q", "k"):
                    qkb[(pj, s)] = const.tile([128, HPC], F32,
                                              tag=f"{pj}b_{s}", name="qb")
                vbb[s] = const.tile([128, QO], BF16, tag=f"vb_{s}", name="vb")

            def const_dmas():
                nc.sync.dma_start(mbt, I["mlpb"][:])
                for s in ("m", "c"):
                    nc.sync.dma_start(vbb[s], bcast(I[f"vb_{s}"][0, :]))

            spq, spk, spv, spmlp = {}, {}, {}, {}
            nhT, rstd_bc, nmr_bc = {}, {}, {}
            with tc.tile_pool(name="nh", bufs=1) as nhp:
                for s in ("m", "c"):
                    nhT[s] = nhp.tile([128, KC, S], BF16, tag=f"nhT_{s}",
                                      name=f"nhT_{s}")
                    rstd_bc[s] = nhp.tile([128, S], BF16, tag="rstd_bc",
                                          name=f"rstd_bc_{s}")
                    nmr_bc[s] = nhp.tile([128, S], BF16, tag="nmr_bc",
                                         name=f"nmr_bc_{s}")
                with tc.tile_pool(name="ph_mlpw", bufs=1) as mw:
                  with (
                    tc.tile_pool(name="ph_n", bufs=1) as pn,
                    tc.tile_pool(name="ph_qkv1", bufs=2) as p1,
                    tc.tile_pool(name="ph_qkv2", bufs=2) as p2,
                    tc.tile_pool(name="ph_qkvw", bufs=2) as pw,
                    tc.tile_pool(name="pstat", bufs=1) as pstat,
                    tc.tile_pool(name="ph_e", bufs=1) as pe,
                    tc.tile_pool(name="psum_qkv", bufs=8, space="PSUM") as ps_qkv,
                  ):
                    PS["p"] = ps_qkv
                    # startup: per stream, ada-norm matvec and layernorm
                    # stats interleaved chunk-by-chunk (DMA-paced, PE ~full)
                    sps = stats_alloc()
                    for kk in range(KC):
                        phase_e_mm("m", kk)
                        stats_chunk("m", kk, sps)
                    phase_e_finish("m")
                    stats_finish("m", *sps)
                    nc.sync.dma_start(outb_cc, I["outb"][:])
                    for s in ("m", "c"):
                        for pj in ("q", "k"):
                            nc.sync.dma_start(qkb[(pj, s)], I[f"{pj}b_{s}"][:])
                    sps = stats_alloc()
                    for kk in range(KC):
                        phase_e_mm("c", kk)
                        stats_chunk("c", kk, sps)
                    phase_e_finish("c")
                    stats_finish("c", *sps)
                    sst_unpack("m")
                    sst_unpack("c")

                    mlpT4 = I["mlpT"][:].rearrange("(kk p) m -> p kk m", p=128)
                    wa0_box = []

                    def mlp_warena(ob):
                        wags = []
                        for g in range(4):
                            wag = mw.tile([128, 6, 384], BF16, tag="wmlpg",
                                          bufs=5, name="wag")
                            nc.sync.dma_start(
                                wag, mlpT4[:, g * 6:(g + 1) * 6,
                                           ob * 384:(ob + 1) * 384])
                            wags.append(wag)
                        return wags

                    def wa0_prefetch():
                        wa0_box.append(mlp_warena(0))

                    def apply_chunk(s, kk):
                        xk = pn.tile([128, S], BF16, tag=f"xk{kk % 3}",
                                     name="xk")
                        nc.sync.dma_start(
                            xk, I[f"xT_{s}"][kk * 128:(kk + 1) * 128, :])
                        t1 = pn.tile([128, S], BF16, tag=f"t1{kk % 2}",
                                     name="t1")
                        nc.vector.tensor_mul(t1, xk, rstd_bc[s])
                        nc.vector.tensor_add(t1, t1, nmr_bc[s])
                        nc.scalar.activation(nhT[s][:, kk, :], t1, AF.Identity,
                                             bias=ss[s][:, kk:kk + 1],
                                             scale=scale1[s][:, kk:kk + 1])

                    def qk_mm_loop(pj, s, hooks):
                        pps = [[PS["p"].tile([128, 512], F32, tag="mm", bufs=6,
                                          name="pps")
                                for _ in range(2)] for _ in range(HPC)]
                        w4 = I[f"w{pj}T_{s}"][:].rearrange(
                            "(kk p) m -> p kk m", p=128)
                        for kk in range(KC):
                            for fn in hooks.get(kk, []):
                                fn()
                            if kk % 2 == 0:
                                wt = pw.tile([128, 2, QO], BF16, tag="wqk",
                                             name="wt")
                                nc.sync.dma_start(wt, w4[:, kk:kk + 2, :])
                            for o in range(HPC):
                                for st in range(2):
                                    nc.tensor.matmul(
                                        pps[o][st],
                                        wt[:, kk % 2, o * 128:(o + 1) * 128],
                                        nhT[s][:, kk, st * 512:(st + 1) * 512],
                                        start=(kk == 0), stop=(kk == KC - 1))
                        return pps

                    def raw_evac(pj, s, pps):
                        rw = pw.tile([128, 2, S], BF16, tag="ropeW", bufs=1,
                                     name="rw")
                        nc.sync.dma_start(
                            rw, I[f"ropeW_{pj}_{s}"][:].rearrange(
                                "(two p) sq -> p two sq", p=128))
                        raws = [rw]
                        for o in range(HPC):
                            raw = p2.tile([128, S], BF16, tag=f"raw{o}",
                                          bufs=1, name="raw")
                            for st in range(2):
                                sl = slice(st * 512, (st + 1) * 512)
                                nc.scalar.activation(
                                    raw[:, sl], pps[o][st], AF.Identity,
                                    bias=qkb[(pj, s)][:, o:o + 1])
                            raws.append(raw)
                        return raws

                    def post_body(pj, s, o, raw, spill, rw,
                                  arena=None):
                        """rms-norm + rope one head (rms weight and the
                        rotate-half sign are folded into cosW/sinW on host;
                        the pair swap itself is a strided SBUF DMA)."""
                        rsw = p1.tile([128, S], BF16, tag="rsw", name="rsw")
                        rsw2 = rsw.rearrange("(a b) s -> a b s", b=2)
                        raw2 = raw.rearrange("(a b) s -> a b s", b=2)
                        nc.sync.dma_start(rsw2[:, 0, :], raw2[:, 1, :])
                        nc.sync.dma_start(rsw2[:, 1, :], raw2[:, 0, :])
                        fin = p1.tile([128, S], BF16, tag="fin", name="fin")
                        nc.vector.tensor_mul(fin, rsw, rw[:, 1, :])
                        t2 = p1.tile([128, S], BF16, tag="t2", name="t2")
                        nc.vector.tensor_mul(t2, raw, rw[:, 0, :])
                        nc.vector.tensor_add(fin, fin, t2)
                        sqh = p1.tile([128, S], BF16, tag="sqh", name="sqh")
                        nc.vector.tensor_mul(sqh, raw, raw)
                        rps = [PS["p"].tile([1, 512], F32, tag="mmT", bufs=2,
                                         name="rps")
                               for _ in range(2)]
                        rsr = p1.tile([1, S], BF16, tag="rsr", name="rsr")
                        for st in range(2):
                            sl = slice(st * 512, (st + 1) * 512)
                            nc.tensor.matmul(rps[st], ones, sqh[:, sl],
                                             start=True, stop=True)
                            nc.scalar.activation(rsr[:, sl], rps[st], AF.Sqrt,
                                                 scale=1.0 / 128,
                                                 bias=epst[:1, :])
                        nc.vector.reciprocal(rsr, rsr)
                        rs_bc = p1.tile([128, S], BF16, tag="rs_bc",
                                        name="rs_bc")
                        mm_bcast(rsr, rs_bc)
                        if arena is not None:
                            nc.vector.tensor_mul(arena[:, o, :], fin, rs_bc)
                        else:
                            nc.vector.tensor_mul(fin, fin, rs_bc)
                            nc.sync.dma_start(
                                spill[o * 128:(o + 1) * 128, :], fin)

                    def v_loop(s, scs, hooks):
                        vps = [PS["p"].tile([128, QO], F32, tag="mm", bufs=6,
                                         name="vps") for _ in scs]
                        w4 = I[f"wvT_{s}"][:].rearrange(
                            "(kk p) m -> p kk m", p=128)
                        for kk in range(KC):
                            for fn in hooks.get(kk, []):
                                fn()
                            if kk % 2 == 0:
                                wt = pw.tile([128, 2, QO], BF16, tag="wqk",
                                             name="wt")
                                nc.sync.dma_start(wt, w4[:, kk:kk + 2, :])
                            for i, sc in enumerate(scs):
                                nc.tensor.matmul(
                                    vps[i], nhT[s][:, kk, sc * 128:(sc + 1) * 128],
                                    wt[:, kk % 2, :],
                                    start=(kk == 0), stop=(kk == KC - 1))
                        return vps

                    def v_evac(s, scs, vps, vsp):
                        for i, sc in enumerate(scs):
                            vt = p1.tile([128, QO], BF16, tag="vt", name="vt")
                            nc.vector.tensor_add(vt, vps[i], vbb[s])
                            nc.sync.dma_start(vsp[sc * 128:(sc + 1) * 128, :], vt)

                    # ---------------- QKV orchestration ------------------
                    # stream m: apply interleaved with q-proj
                    hooks = {kk: [lambda kk=kk: apply_chunk("m", kk)]
                             for kk in range(KC)}
                    pps_q = qk_mm_loop("q", "m", hooks)
                    spq["m"] = dram.tile([QO, S], BF16, tag="sp_q_m",
                                         name="spq_m")
                    raws_q = raw_evac("q", "m", pps_q)

                    # k(m): big-const DMAs, MLP arena prefetch and q(m)
                    # head-posts all hooked into the matmul loop
                    hooks = {5 + 6 * o: [lambda o=o: post_body(
                        "q", "m", o, raws_q[o + 1], None, raws_q[0],
                        arena=qm_arena)] for o in range(HPC)}
                    hooks[1] = [const_dmas]
                    hooks[3] = [wa0_prefetch]
                    pps_k = qk_mm_loop("k", "m", hooks)
                    spk["m"] = dram.tile([QO, S], BF16, tag="sp_k_m",
                                         name="spk_m")
                    raws_k = raw_evac("k", "m", pps_k)

                    # v(m) part A carries stream c's stats; part B carries
                    # stream c's ada-norm matvec and the k(m) head-posts
                    spv["m"] = dram.tile([S, QO], BF16, tag="sp_v_m",
                                         name="spv_m")
                    hooks = {kk: [lambda kk=kk: apply_chunk("c", kk)]
                             for kk in range(KC)}
                    vps_a = v_loop("m", range(6), hooks)
                    v_evac("m", range(6), vps_a, spv["m"])
                    hooks = {5 + 6 * o: [lambda o=o: post_body(
                        "k", "m", o, raws_k[o + 1], spk["m"], raws_k[0])]
                        for o in range(HPC)}
                    vps_b = v_loop("m", range(6, 8), hooks)
                    v_evac("m", range(6, 8), vps_b, spv["m"])

                    # stream c (nhT_c already applied under v(m) part A)
                    pps_q = qk_mm_loop("q", "c", {})
                    spq["c"] = dram.tile([QO, S], BF16, tag="sp_q_c",
                                         name="spq_c")
                    raws_q = raw_evac("q", "c", pps_q)

                    hooks = {5 + 6 * o: [lambda o=o: post_body(
                        "q", "c", o, raws_q[o + 1], spq["c"], raws_q[0])]
                        for o in range(HPC)}
                    pps_k = qk_mm_loop("k", "c", hooks)
                    spk["c"] = dram.tile([QO, S], BF16, tag="sp_k_c",
                                         name="spk_c")
                    raws_k = raw_evac("k", "c", pps_k)

                    spv["c"] = dram.tile([S, QO], BF16, tag="sp_v_c",
                                         name="spv_c")
                    vps_a = v_loop("c", range(6), {})
                    v_evac("c", range(6), vps_a, spv["c"])
                    hooks = {5 + 6 * o: [lambda o=o: post_body(
                        "k", "c", o, raws_k[o + 1], spk["c"], raws_k[0])]
                        for o in range(HPC)}
                    vps_b = v_loop("c", range(6, 8), hooks)
                    v_evac("c", range(6, 8), vps_b, spv["c"])


                  # ---------- phase MLP (gelu act table); weight arena is
                  # loaded once per ob and shared by both streams -----------
                  for s in ("m", "c"):
                      spmlp[s] = dram.tile([MLPC, S], BF16, tag=f"sp_mlp_{s}",
                                           name=f"sp_mlp_{s}")
                  with (
                      tc.tile_pool(name="ph_mlpo", bufs=2) as mo,
                      tc.tile_pool(name="psum_mlp", bufs=8, space="PSUM") as ps_mlp,
                  ):
                    PS["p"] = ps_mlp
                    for ob in range(4):
                        wa = wa0_box[0] if ob == 0 else mlp_warena(ob)
                        for s in ("m", "c"):
                            mps = [[PS["p"].tile([128, 512], F32, tag="mm",
                                              bufs=8, name="mps")
                                    for _ in range(2)] for o4 in range(3)]
                            for kk in range(KC):
                                for o4 in range(3):
                                    for st in range(2):
                                        nc.tensor.matmul(
                                            mps[o4][st],
                                            wa[kk // 6][:, kk % 6,
                                                        o4 * 128:(o4 + 1) * 128],
                                            nhT[s][:, kk, st * 512:(st + 1) * 512],
                                            start=(kk == 0), stop=(kk == KC - 1))
                            for o4 in range(3):
                                o = ob * 3 + o4
                                mt = mo.tile([128, S], BF16, tag="mt")
                                for st in range(2):
                                    sl = slice(st * 512, (st + 1) * 512)
                                    nc.scalar.activation(mt[:, sl], mps[o4][st],
                                                         AF.Gelu_apprx_tanh,
                                                         bias=mbt[:, o:o + 1])
                                nc.sync.dma_start(
                                    spmlp[s][o * 128:(o + 1) * 128, :], mt)

            # ---------------- phase ATTN ----------------------------------
            with tc.tile_pool(name="attn_out", bufs=1) as ao:
              with (
                tc.tile_pool(name="attn_qkv", bufs=1) as aq,
                tc.tile_pool(name="attn_wk", bufs=3) as awk,
                tc.tile_pool(name="attn_w1", bufs=2) as aw1,
                tc.tile_pool(name="psum_att", bufs=8, space="PSUM") as ps_att,
              ):
                PS["p"] = ps_att
                qm = qm_arena
                am = ao.tile([128, HPC, S], BF16, tag="am")
                ac = ao.tile([128, HPC, S], BF16, tag="ac")
                att_out = {"m": am, "c": ac}
                kt = vt = None
                for attn, (qs, ks, acc_t, fresh) in (
                    ("main", ("m", "m", am, True)),
                    ("ctrl", ("c", "c", ac, True)),
                    ("cross", ("m", "c", am, False)),
                ):
                    if attn != "cross":
                        kt = aq.tile([128, HPC, S], BF16, tag="kt")
                        vt = aq.tile([128, 8, QO], BF16, tag="vt")
                        nc.sync.dma_start(
                            kt, spk[ks][:].rearrange("(h p) s -> p h s", p=128))
                        nc.sync.dma_start(
                            vt, spv[ks][:].rearrange("(sc p) q -> p sc q", p=128))
                    if attn == "ctrl":
                        qt = aq.tile([128, HPC, S], BF16, tag="qc")
                        nc.sync.dma_start(
                            qt, spq["c"][:].rearrange("(h p) s -> p h s", p=128))
                    else:
                        qt = qm
                    for h in range(HPC):
                        av_ps = [PS["p"].tile([128, 512], F32, tag="mmA",
                                              bufs=3, name="av_ps")
                                 for _ in range(2)]
                        esum = aw1.tile([128, S], BF16, tag="esum")
                        for kv in range(8):
                            s_ps = PS["p"].tile([128, 1024], F32, tag="mmL",
                                                bufs=2, name="s_ps")
                            et = awk.tile([128, S], BF16, tag="exp")
                            for st in range(2):
                                sl = slice(st * 512, (st + 1) * 512)
                                nc.tensor.matmul(s_ps[:, sl],
                                                 kt[:, h, kv * 128:(kv + 1) * 128],
                                                 qt[:, h, sl], start=True, stop=True)
                            nc.scalar.activation(et, s_ps, AF.Exp,
                                                 scale=INV_SQRT_HD)
                            for st in range(2):
                                sl = slice(st * 512, (st + 1) * 512)
                                nc.tensor.matmul(av_ps[st],
                                                 vt[:, kv, h * 128:(h + 1) * 128],
                                                 et[:, sl], start=(kv == 0),
                                                 stop=(kv == 7))
                            if kv == 0:
                                nc.vector.tensor_scalar_add(esum, et, 0.0)
                            else:
                                nc.vector.tensor_add(esum, esum, et)
                        d_ps = [PS["p"].tile([1, 512], F32, tag="mmT", bufs=1, name="d_ps")
                                for _ in range(2)]
                        den_row = aw1.tile([1, S], BF16, tag="den")
                        for st in range(2):
                            sl = slice(st * 512, (st + 1) * 512)
                            nc.tensor.matmul(d_ps[st], ones, esum[:, sl],
                                             start=True, stop=True)
                            nc.vector.reciprocal(den_row[:, sl], d_ps[st])
                        den_bc = aw1.tile([128, S], BF16, tag="den_bc")
                        for st in range(2):
                            sl = slice(st * 512, (st + 1) * 512)
                            bp = PS["p"].tile([128, 512], F32, tag="mmT",
                                              bufs=1, name="bc_ps2")
                            nc.tensor.matmul(bp, onesrow, den_row[:, sl],
                                             start=True, stop=True)
                            nc.vector.tensor_scalar_add(den_bc[:, sl], bp, 0.0)
                        for st in range(2):
                            sl = slice(st * 512, (st + 1) * 512)
                            if fresh:
                                nc.vector.tensor_mul(acc_t[:, h, sl], av_ps[st],
                                                     den_bc[:, sl])
                            else:
                                cr = aw1.tile([128, 512], BF16, tag="crs")
                                nc.vector.tensor_mul(cr, av_ps[st], den_bc[:, sl])
                                nc.vector.tensor_add(acc_t[:, h, sl],
                                                     acc_t[:, h, sl], cr)

               # ---------------- phase OUT-PROJ --------------------------
              with (
                  tc.tile_pool(name="hmlp", bufs=1) as hm,
                  tc.tile_pool(name="ow", bufs=2) as ow,
                  tc.tile_pool(name="fin", bufs=4) as fp,
                  tc.tile_pool(name="psum_out", bufs=8, space="PSUM") as ps_out,
              ):
                  PS["p"] = ps_out
                  mlp_sb = {}
                  for s in ("m", "c"):
                      mt = hm.tile([128, MLPC // 128, S], BF16, tag=f"hmlp_{s}")
                      for e in range(MLPC // 128):
                          nc.sync.dma_start(mt[:, e, :],
                                            spmlp[s][e * 128:(e + 1) * 128, :])
                      mlp_sb[s] = mt
                  for oc in range(KC):
                      warena = ow.tile([128, HKC, 128], BF16, tag="warena")
                      nc.sync.dma_start(
                          warena[:].rearrange("p kk c -> p (kk c)"),
                          I["outT"][oc * 128:(oc + 1) * 128, :])
                      for s in ("m", "c"):
                          xs = fp.tile([128, S], BF16, tag="xs")
                          nc.sync.dma_start(
                              xs, I[f"xT_{s}"][oc * 128:(oc + 1) * 128, :])
                          nc.vector.tensor_scalar_mul(xs, xs, 1.0 / NCORES)
                          ops_t = [PS["p"].tile([128, 512], F32, tag="mm", bufs=6, name="ops_t")
                                   for _ in range(2)]
                          for kk in range(HKC):
                              rh = (att_out[s][:, kk, :] if kk < HPC
                                    else mlp_sb[s][:, kk - HPC, :])
                              for st in range(2):
                                  nc.tensor.matmul(
                                      ops_t[st], warena[:, kk, :],
                                      rh[:, st * 512:(st + 1) * 512],
                                      start=(kk == 0), stop=(kk == HKC - 1))
                          tg = fp.tile([128, S], BF16, tag="tg")
                          for st in range(2):
                              sl = slice(st * 512, (st + 1) * 512)
                              nc.scalar.activation(tg[:, sl], ops_t[st],
                                                   AF.Identity,
                                                   scale=g_sb[s][:, oc:oc + 1],
                                                   bias=ob8[s][:, oc:oc + 1])
                          nc.vector.tensor_add(tg, tg, xs)
                          nc.sync.dma_start(
                              OUT[s][oc * 128:(oc + 1) * 128, :], tg)
    nc.compile()
    return nc


_NC_CACHE = []


def _get_nc():
    if not _NC_CACHE:
        _NC_CACHE.append(build_nc())
    return _NC_CACHE[0]


def _prep_core_inputs(inputs, c):
    f = np.float32
    bf = ml_dtypes.bfloat16
    T = lambda a: np.ascontiguousarray(np.asarray(a, f).T)
    Tb = lambda a: np.ascontiguousarray(np.asarray(a, f).T.astype(bf))
    C = lambda a: np.ascontiguousarray(np.asarray(a, f))
    hs = np.asarray(inputs["hidden_states"], f)[0]
    hc = np.asarray(inputs["hidden_states_control"], f)[0]
    m = {}
    for s, x, te, nw, nb in (
        ("m", hs, inputs["temb"], inputs["norm_w"], inputs["norm_b"]),
        ("c", hc, inputs["temb_control"], inputs["normc_w"], inputs["normc_b"]),
    ):
        m[f"xT_{s}"] = Tb(x)
        tev = np.asarray(te, f)[0]
        tev = tev / (1.0 + np.exp(-tev))  # host-side silu
        m[f"temb_{s}"] = np.ascontiguousarray(
            tev.reshape(KC, 128).T.astype(bf))
        m[f"normT_{s}"] = Tb(np.asarray(nw, f)[c * ES:(c + 1) * ES, :])
        m[f"nb_{s}"] = np.ascontiguousarray(
            np.asarray(nb, f)[c * ES:(c + 1) * ES].reshape(1, ES).astype(bf))
    cosT = np.asarray(inputs["rope_cos"], f).T  # [128, S]
    sinT = np.asarray(inputs["rope_sin"], f).T
    swap = np.arange(128) ^ 1
    for s, pre in (("m", ""), ("c", "c")):
        for w in ("q", "k", "v"):
            W = np.asarray(inputs[f"{w}{pre}_w"], f)
            m[f"w{w}T_{s}"] = Tb(W[c * QO:(c + 1) * QO, :])
        for w in ("q", "k"):
            b = np.asarray(inputs[f"{w}{pre}_b"], f)[c * QO:(c + 1) * QO]
            m[f"{w}b_{s}"] = C(b.reshape(HPC, 128).T)
            rw = np.asarray(inputs[f"rms_{w}{pre}"], f)  # [128]
            sgn = np.where(np.arange(128) % 2 == 0, -1.0, 1.0).astype(f)
            m[f"ropeW_{w}_{s}"] = np.ascontiguousarray(np.concatenate(
                [cosT * rw[:, None], sinT * (rw[swap] * sgn)[:, None]],
                axis=0).astype(bf))
        m[f"vb_{s}"] = np.ascontiguousarray(
            np.asarray(inputs[f"v{pre}_b"], f)[c * QO:(c + 1) * QO]
            .reshape(1, QO).astype(bf))
    m["mlpT"] = Tb(np.asarray(inputs["mlp_w"], f)[c * MLPC:(c + 1) * MLPC, :])
    m["mlpb"] = C(np.asarray(inputs["mlp_b"], f)[c * MLPC:(c + 1) * MLPC]
                  .reshape(MLPC // 128, 128).T)
    ow = np.asarray(inputs["out_w"], f)
    W = np.concatenate(
        [ow[:, c * QO:(c + 1) * QO], ow[:, D + c * MLPC:D + (c + 1) * MLPC]],
        axis=1)  # [3072 d, 1920 h]
    # blocked: [oc, p, kk, col] = W[oc*128+col, kk*128+p]
    W4 = W.reshape(KC, 128, HKC, 128).transpose(0, 3, 2, 1)
    m["outT"] = np.ascontiguousarray(
        W4.reshape(KC * 128, HKC * 128).astype(bf))
    m["outb"] = C(np.asarray(inputs["out_b"], f).reshape(KC, 128).T)
    R = np.zeros((128, 128), f)
    for i in range(64):
        R[2 * i, 2 * i + 1] = -1.0
        R[2 * i + 1, 2 * i] = 1.0
    m["rotT"] = np.ascontiguousarray(R.T.astype(bf))
    return m


def run_cores(inputs, trace=False):
    nc = _get_nc()
    in_maps = [_prep_core_inputs(inputs, c) for c in range(NCORES)]
    res = run_bass_kernel_spmd(nc, in_maps, list(range(NCORES)), trace=trace)
    h = np.sum([np.asarray(r["out_m"], np.float64) for r in res.results], axis=0)
    hc = np.sum([np.asarray(r["out_c"], np.float64) for r in res.results], axis=0)
    h = np.ascontiguousarray(h.T.astype(np.float32)).reshape(1, S, D)
    hc = np.ascontiguousarray(hc.T.astype(np.float32)).reshape(1, S, D)
    return (h, hc), res


def kernel(**inputs):
    out, _ = run_cores(inputs, trace=False)
    return out

